# revision 9
# baseline (speedup 1.0000x reference)
"""Trainium2 Bass kernel for nn_Clustering_80900003987951 (vq_codebook).

Math (reference):
  x: [B=128, S=128, F=64, 1], centroids: [1, K=64, S=128, F=64]
  d2[b,k,s] = sum_f (x[b,s,f] - c[k,s,f])^2
  dist[b,k] = sum_s sqrt(d2[b,k,s])
  q = (1 + dist^2/2)^-3 / 2, normalized over k           -> [B, K]

Strategy: data-parallel over batch across 8 cores (B_loc=16), centroids
replicated. All transposes + augmentation are done host-side:
  XT [66, S*16]: rows 0-63 = x^T (F on partitions), row 64 = 1, row 65 = |x|^2
  CT [66, S*64]: rows 0-63 = -2*c^T,  row 64 = |c|^2,  row 65 = 1
so one fp32 matmul per s gives the complete d2[b,k] tile in PSUM:
  d2 = XT_s^T @ CT_s = -2<x,c> + |c|^2 + |x|^2.
128 matmuls are packed 4-wide with tile_position column tiling; sqrt on
ACT per PSUM bank; s-reduction and the q tail on DVE.
"""

import numpy as np

B, K, S, F = 128, 64, 128, 64
NCORES = 8
BLOC = B // NCORES          # 16
CP = F + 2                  # 66 contraction rows (data + cn + xn aug)
NCHUNK = 4                  # psum banks / s-chunks
S_CH = S // NCHUNK          # 32 s per chunk
TBLK = S // 4               # 32 column blocks of 64 in psum

IN_DT = "float16"   # dtype of the streamed xt/ct operands ("float32"/"float16")

_CACHE = {}


def _build_nc():
    import concourse.bacc as bacc
    import concourse.tile as tile
    from concourse import mybir
    import concourse.bass as bass

    f32 = mybir.dt.float32
    fin = getattr(mybir.dt, IN_DT)
    nc = bacc.Bacc("TRN2", target_bir_lowering=False, debug=False)

    xt_d = nc.dram_tensor("xt", [CP, S * BLOC], fin, kind="ExternalInput")
    ct_d = nc.dram_tensor("ct", [CP, S * K], fin, kind="ExternalInput")
    # strip[p, b] = 1 if p % 32 == b else 0 — matmul-based partition-strip sum
    st_d = nc.dram_tensor("strip", [128, BLOC], f32, kind="ExternalInput")
    q_d = nc.dram_tensor("q", [BLOC, K], f32, kind="ExternalOutput")

    with tile.TileContext(nc) as tc:
        with (
            tc.tile_pool(name="xt", bufs=1) as xt_pool,
            tc.tile_pool(name="ct", bufs=NCHUNK) as ct_pool,
            tc.tile_pool(name="psum", bufs=1, space="PSUM") as psum_pool,
            tc.tile_pool(name="dist", bufs=1) as dist_pool,
            tc.tile_pool(name="work", bufs=2) as work_pool,
            tc.tile_pool(name="tail", bufs=1) as tail_pool,
        ):
            xt_t = xt_pool.tile([CP, S * BLOC], f32)
            nc.sync.dma_start(out=xt_t[:], in_=xt_d.ap())
            st_t = xt_pool.tile([128, BLOC], f32)
            nc.sync.dma_start(out=st_t[:], in_=st_d.ap())

            # d2 landing zone: col block t = s//4 (64 wide), partition strip
            # 32*(s%4) + b.  Chunk c covers s in [32c, 32c+32) = bank c.
            psum = psum_pool.tile([128, TBLK * K // 32 * 32], f32)  # [128, 2048]
            # Garbage partitions (16-31 etc.) are never matmul-written; zero
            # them once so the bank-wide sqrt reads defined data.
            nc.vector.memset(psum[:], 0.0)

            # dist, k-major: [128, K, TBLK] so the s-reduction is unit-stride.
            dist_t = dist_pool.tile([128, K, TBLK], f32)
            # per-chunk partial s-sums, concatenated along free dim
            part4 = tail_pool.tile([128, NCHUNK, K], f32)

            ct_tiles = []
            for c in range(NCHUNK):
                ct_t = ct_pool.tile([CP, S_CH * K], f32, tag="ct")
                nc.sync.dma_start(
                    out=ct_t[:], in_=ct_d.ap()[:, c * S_CH * K:(c + 1) * S_CH * K]
                )
                ct_tiles.append(ct_t)

            for c in range(NCHUNK):
                ct_t = ct_tiles[c]
                for u in range(S_CH):
                    s = c * S_CH + u
                    j = s % 4
                    t = s // 4
                    nc.tensor.matmul(
                        psum[32 * j:32 * j + BLOC, t * K:(t + 1) * K],
                        lhsT=xt_t[:, s * BLOC:(s + 1) * BLOC],
                        rhs=ct_t[:, u * K:(u + 1) * K],
                        start=True,
                        stop=True,
                        tile_position=(0, 32 * j),
                    )
                # sqrt of bank c: psum cols [512c, 512c+512) hold t in
                # [8c, 8c+8). Write k-major into dist_t[:, :, 8c:8c+8].
                t0 = 8 * c
                out_ap = dist_t[:, :, t0:t0 + 8].rearrange("p k t -> p t k")
                nc.scalar.activation(
                    out_ap,
                    psum[:, 512 * c:512 * c + 512],
                    mybir.ActivationFunctionType.Sqrt,
                )
                nc.vector.tensor_reduce(
                    part4[:, c, :],
                    dist_t[:, :, t0:t0 + 8],
                    axis=mybir.AxisListType.X,
                    op=mybir.AluOpType.add,
                )

            # Strip+partition sum via matmul: out[b, (c,k)] = sum_p strip[p,b]
            # * part4[p, c, k] = sum_j part4[32j+b, c, k].
            dsum_ps = psum_pool.tile([BLOC, NCHUNK * K], f32)
            nc.tensor.matmul(
                dsum_ps[:],
                lhsT=st_t[:],
                rhs=part4[:].rearrange("p c k -> p (c k)"),
                start=True,
                stop=True,
            )
            # dsum[b,k] = sum_c dsum_ps[b, c*K+k]
            dsum = tail_pool.tile([BLOC, K], f32)
            nc.vector.tensor_reduce(
                dsum[:],
                dsum_ps[:].rearrange("p (c k) -> p k c", c=NCHUNK),
                axis=mybir.AxisListType.X,
                op=mybir.AluOpType.add,
            )

            # q tail: w = 1 + dsum^2/2; r = 1/w; r3 = r^3; q = r3/sum_k r3
            w = tail_pool.tile([BLOC, K], f32)
            nc.vector.tensor_tensor(w[:], dsum[:], dsum[:], op=mybir.AluOpType.mult)
            nc.vector.tensor_scalar(
                w[:], w[:], 0.5, 1.0,
                op0=mybir.AluOpType.mult, op1=mybir.AluOpType.add,
            )
            r = tail_pool.tile([BLOC, K], f32)
            nc.vector.reciprocal(r[:], w[:])
            r2 = tail_pool.tile([BLOC, K], f32)
            nc.vector.tensor_tensor(r2[:], r[:], r[:], op=mybir.AluOpType.mult)
            r3 = tail_pool.tile([BLOC, K], f32)
            nc.vector.tensor_tensor(r3[:], r2[:], r[:], op=mybir.AluOpType.mult)
            ssum = tail_pool.tile([BLOC, 1], f32)
            nc.vector.tensor_reduce(
                ssum[:], r3[:], axis=mybir.AxisListType.X, op=mybir.AluOpType.add
            )
            rs = tail_pool.tile([BLOC, 1], f32)
            nc.vector.reciprocal(rs[:], ssum[:])
            qt = tail_pool.tile([BLOC, K], f32)
            nc.vector.tensor_scalar(
                qt[:], r3[:], rs[:], None, op0=mybir.AluOpType.mult
            )
            nc.sync.dma_start(out=q_d.ap(), in_=qt[:])

    nc.compile()
    return nc


def _prep_inputs(x, centroids):
    """Host-side shard + transpose + augmentation. Returns in_maps list."""
    x = np.ascontiguousarray(np.asarray(x, dtype=np.float32)).reshape(B, S, F)
    c = np.ascontiguousarray(np.asarray(centroids, dtype=np.float32)).reshape(K, S, F)

    # CT [66, S*K], cols s*K + k
    ct = np.empty((CP, S * K), dtype=np.float32)
    ct[:F] = (-2.0 * c).transpose(2, 1, 0).reshape(F, S * K)
    ct[F] = ((c * c).sum(-1, dtype=np.float32).T).reshape(S * K)
    ct[F + 1] = 1.0

    # strip-sum selector: strip[p, b] = 1 if p % 32 == b else 0
    strip = np.zeros((128, BLOC), dtype=np.float32)
    for p in range(128):
        if p % 32 < BLOC:
            strip[p, p % 32] = 1.0

    in_maps = []
    for i in range(NCORES):
        xs = x[i * BLOC:(i + 1) * BLOC]          # [16, S, F]
        xt = np.empty((CP, S * BLOC), dtype=np.float32)
        xt[:F] = xs.transpose(2, 1, 0).reshape(F, S * BLOC)
        xt[F] = 1.0
        xt[F + 1] = ((xs * xs).sum(-1, dtype=np.float32).T).reshape(S * BLOC)
        in_maps.append({"xt": xt, "ct": ct, "strip": strip})
    return in_maps


def kernel(x, centroids):
    from concourse.bass_utils import run_bass_kernel_spmd

    if "nc" not in _CACHE:
        _CACHE["nc"] = _build_nc()
    nc = _CACHE["nc"]

    in_maps = _prep_inputs(x, centroids)
    res = run_bass_kernel_spmd(nc, in_maps, core_ids=list(range(NCORES)))
    out = np.concatenate([res.results[i]["q"] for i in range(NCORES)], axis=0)
    return out.astype(np.float32)


# revision 13
# speedup vs baseline: 1.1647x; 1.1647x over previous
"""Trainium2 Bass kernel for nn_Clustering_80900003987951 (vq_codebook).

Math (reference):
  x: [B=128, S=128, F=64, 1], centroids: [1, K=64, S=128, F=64]
  d2[b,k,s] = sum_f (x[b,s,f] - c[k,s,f])^2
  dist[b,k] = sum_s sqrt(d2[b,k,s])
  q = (1 + dist^2/2)^-3 / 2, normalized over k           -> [B, K]

Strategy: data-parallel over batch across 8 cores (B_loc=16), centroids
replicated. All transposes + augmentation are done host-side:
  XT [66, S*16]: rows 0-63 = x^T (F on partitions), row 64 = 1, row 65 = |x|^2
  CT [66, S*64]: rows 0-63 = -2*c^T,  row 64 = |c|^2,  row 65 = 1
so one fp32 matmul per s gives the complete d2[b,k] tile in PSUM:
  d2 = XT_s^T @ CT_s = -2<x,c> + |c|^2 + |x|^2.
128 matmuls are packed 4-wide with tile_position column tiling; sqrt on
ACT per PSUM bank; s-reduction and the q tail on DVE.
"""

import numpy as np

B, K, S, F = 128, 64, 128, 64
NCORES = 8
BLOC = B // NCORES          # 16
CP = F + 2                  # 66 contraction rows (data + cn + xn aug)
NCHUNK = 4                  # psum banks / s-chunks
S_CH = S // NCHUNK          # 32 s per chunk
TBLK = S // 4               # 32 column blocks of 64 in psum

IN_DT = "float16"   # dtype of the streamed xt/ct operands ("float32"/"float16")

_CACHE = {}


def _build_nc():
    import concourse.bacc as bacc
    import concourse.tile as tile
    from concourse import mybir
    import concourse.bass as bass

    f32 = mybir.dt.float32
    fin = getattr(mybir.dt, IN_DT)
    nc = bacc.Bacc("TRN2", target_bir_lowering=False, debug=False)

    xt_d = nc.dram_tensor("xt", [CP, S * BLOC], fin, kind="ExternalInput")
    ct_d = nc.dram_tensor("ct", [CP, S * K], fin, kind="ExternalInput")
    # strip[p, b] = 1 if p % 32 == b else 0 — matmul-based partition-strip sum
    st_d = nc.dram_tensor("strip", [128, BLOC], f32, kind="ExternalInput")
    q_d = nc.dram_tensor("q", [BLOC, K], f32, kind="ExternalOutput")

    with tile.TileContext(nc) as tc:
        with (
            tc.tile_pool(name="xt", bufs=1) as xt_pool,
            tc.tile_pool(name="ct", bufs=NCHUNK) as ct_pool,
            tc.tile_pool(name="psum", bufs=1, space="PSUM") as psum_pool,
            tc.tile_pool(name="dist", bufs=1) as dist_pool,
            tc.tile_pool(name="work", bufs=2) as work_pool,
            tc.tile_pool(name="tail", bufs=1) as tail_pool,
        ):
            xt_t = xt_pool.tile([CP, S * BLOC], fin)
            nc.sync.dma_start(out=xt_t[:], in_=xt_d.ap())
            st_t = xt_pool.tile([128, BLOC], f32)
            nc.sync.dma_start(out=st_t[:], in_=st_d.ap())

            # d2 landing zone: col block t = s//4 (64 wide), partition strip
            # 32*(s%4) + b.  Chunk c covers s in [32c, 32c+32) = bank c.
            psum = psum_pool.tile([128, TBLK * K // 32 * 32], f32)  # [128, 2048]
            # Garbage partitions (16-31 etc.) are never matmul-written; zero
            # them once so the bank-wide sqrt reads defined data.
            nc.vector.memset(psum[:], 0.0)

            # dist, k-major: [128, K, TBLK] so the s-reduction is unit-stride.
            dist_t = dist_pool.tile([128, K, TBLK], f32)
            # per-chunk partial s-sums, concatenated along free dim
            part4 = tail_pool.tile([128, NCHUNK, K], f32)

            ct_tiles = []
            for c in range(NCHUNK):
                ct_t = ct_pool.tile([CP, S_CH * K], fin, tag="ct")
                nc.sync.dma_start(
                    out=ct_t[:], in_=ct_d.ap()[:, c * S_CH * K:(c + 1) * S_CH * K]
                )
                ct_tiles.append(ct_t)

            for c in range(NCHUNK):
                ct_t = ct_tiles[c]
                for u in range(S_CH):
                    s = c * S_CH + u
                    j = s % 4
                    t = s // 4
                    nc.tensor.matmul(
                        psum[32 * j:32 * j + BLOC, t * K:(t + 1) * K],
                        lhsT=xt_t[:, s * BLOC:(s + 1) * BLOC],
                        rhs=ct_t[:, u * K:(u + 1) * K],
                        start=True,
                        stop=True,
                        tile_position=(0, 32 * j),
                    )
                # sqrt of bank c: psum cols [512c, 512c+512) hold t in
                # [8c, 8c+8). Write k-major into dist_t[:, :, 8c:8c+8].
                t0 = 8 * c
                out_ap = dist_t[:, :, t0:t0 + 8].rearrange("p k t -> p t k")
                nc.scalar.activation(
                    out_ap,
                    psum[:, 512 * c:512 * c + 512],
                    mybir.ActivationFunctionType.Sqrt,
                )
                nc.vector.tensor_reduce(
                    part4[:, c, :],
                    dist_t[:, :, t0:t0 + 8],
                    axis=mybir.AxisListType.X,
                    op=mybir.AluOpType.add,
                )

            # Strip+partition sum via matmul: out[b, (c,k)] = sum_p strip[p,b]
            # * part4[p, c, k] = sum_j part4[32j+b, c, k].
            dsum_ps = psum_pool.tile([BLOC, NCHUNK * K], f32)
            nc.tensor.matmul(
                dsum_ps[:],
                lhsT=st_t[:],
                rhs=part4[:].rearrange("p c k -> p (c k)"),
                start=True,
                stop=True,
            )
            # dsum[b,k] = sum_c dsum_ps[b, c*K+k]
            dsum = tail_pool.tile([BLOC, K], f32)
            nc.vector.tensor_reduce(
                dsum[:],
                dsum_ps[:].rearrange("p (c k) -> p k c", c=NCHUNK),
                axis=mybir.AxisListType.X,
                op=mybir.AluOpType.add,
            )

            # q tail: w = 1 + dsum^2/2; r = 1/w; r3 = r^3; q = r3/sum_k r3
            w = tail_pool.tile([BLOC, K], f32)
            nc.vector.tensor_tensor(w[:], dsum[:], dsum[:], op=mybir.AluOpType.mult)
            nc.vector.tensor_scalar(
                w[:], w[:], 0.5, 1.0,
                op0=mybir.AluOpType.mult, op1=mybir.AluOpType.add,
            )
            r = tail_pool.tile([BLOC, K], f32)
            nc.vector.reciprocal(r[:], w[:])
            r2 = tail_pool.tile([BLOC, K], f32)
            nc.vector.tensor_tensor(r2[:], r[:], r[:], op=mybir.AluOpType.mult)
            r3 = tail_pool.tile([BLOC, K], f32)
            nc.vector.tensor_tensor(r3[:], r2[:], r[:], op=mybir.AluOpType.mult)
            ssum = tail_pool.tile([BLOC, 1], f32)
            nc.vector.tensor_reduce(
                ssum[:], r3[:], axis=mybir.AxisListType.X, op=mybir.AluOpType.add
            )
            rs = tail_pool.tile([BLOC, 1], f32)
            nc.vector.reciprocal(rs[:], ssum[:])
            qt = tail_pool.tile([BLOC, K], f32)
            nc.vector.tensor_scalar(
                qt[:], r3[:], rs[:], None, op0=mybir.AluOpType.mult
            )
            nc.sync.dma_start(out=q_d.ap(), in_=qt[:])

    nc.compile()
    return nc


def _prep_inputs(x, centroids):
    """Host-side shard + transpose + augmentation. Returns in_maps list."""
    in_np = np.dtype(IN_DT)
    x = np.ascontiguousarray(np.asarray(x, dtype=np.float32)).reshape(B, S, F)
    c = np.ascontiguousarray(np.asarray(centroids, dtype=np.float32)).reshape(K, S, F)

    # CT [66, S*K], cols s*K + k
    ct = np.empty((CP, S * K), dtype=in_np)
    ct[:F] = (-2.0 * c).transpose(2, 1, 0).reshape(F, S * K)
    ct[F] = ((c * c).sum(-1, dtype=np.float32).T).reshape(S * K)
    ct[F + 1] = 1.0

    # strip-sum selector: strip[p, b] = 1 if p % 32 == b else 0
    strip = np.zeros((128, BLOC), dtype=np.float32)
    for p in range(128):
        if p % 32 < BLOC:
            strip[p, p % 32] = 1.0

    in_maps = []
    for i in range(NCORES):
        xs = x[i * BLOC:(i + 1) * BLOC]          # [16, S, F]
        xt = np.empty((CP, S * BLOC), dtype=in_np)
        xt[:F] = xs.transpose(2, 1, 0).reshape(F, S * BLOC)
        xt[F] = 1.0
        xt[F + 1] = ((xs * xs).sum(-1, dtype=np.float32).T).reshape(S * BLOC)
        in_maps.append({"xt": xt, "ct": ct, "strip": strip})
    return in_maps


def kernel(x, centroids):
    from concourse.bass_utils import run_bass_kernel_spmd

    if "nc" not in _CACHE:
        _CACHE["nc"] = _build_nc()
    nc = _CACHE["nc"]

    in_maps = _prep_inputs(x, centroids)
    res = run_bass_kernel_spmd(nc, in_maps, core_ids=list(range(NCORES)))
    out = np.concatenate([res.results[i]["q"] for i in range(NCORES)], axis=0)
    return out.astype(np.float32)


# revision 16
# speedup vs baseline: 1.2323x; 1.0580x over previous
"""Trainium2 Bass kernel for nn_Clustering_80900003987951 (vq_codebook).

Math (reference):
  x: [B=128, S=128, F=64, 1], centroids: [1, K=64, S=128, F=64]
  d2[b,k,s] = sum_f (x[b,s,f] - c[k,s,f])^2
  dist[b,k] = sum_s sqrt(d2[b,k,s])
  q = (1 + dist^2/2)^-3 / 2, normalized over k           -> [B, K]

Strategy: data-parallel over batch across 8 cores (B_loc=16), centroids
replicated. All transposes + augmentation are done host-side:
  XT [66, S*16]: rows 0-63 = x^T (F on partitions), row 64 = 1, row 65 = |x|^2
  CT [66, S*64]: rows 0-63 = -2*c^T,  row 64 = |c|^2,  row 65 = 1
so one fp32 matmul per s gives the complete d2[b,k] tile in PSUM:
  d2 = XT_s^T @ CT_s = -2<x,c> + |c|^2 + |x|^2.
128 matmuls are packed 4-wide with tile_position column tiling; sqrt on
ACT per PSUM bank; s-reduction and the q tail on DVE.
"""

import numpy as np

B, K, S, F = 128, 64, 128, 64
NCORES = 8
BLOC = B // NCORES          # 16
CP = F + 2                  # 66 contraction rows (data + cn + xn aug)
NCHUNK = 4                  # psum banks / s-chunks
S_CH = S // NCHUNK          # 32 s per chunk
TBLK = S // 4               # 32 column blocks of 64 in psum

IN_DT = "float16"   # dtype of the streamed xt/ct operands ("float32"/"float16")

_CACHE = {}


def _build_nc():
    import concourse.bacc as bacc
    import concourse.tile as tile
    from concourse import mybir
    import concourse.bass as bass

    f32 = mybir.dt.float32
    fin = getattr(mybir.dt, IN_DT)
    nc = bacc.Bacc("TRN2", target_bir_lowering=False, debug=False)

    xt_d = nc.dram_tensor("xt", [CP, S * BLOC], fin, kind="ExternalInput")
    ct_d = nc.dram_tensor("ct", [CP, S * K], fin, kind="ExternalInput")
    # strip[p, b] = 1 if p % 32 == b else 0 — matmul-based partition-strip sum
    st_d = nc.dram_tensor("strip", [128, BLOC], f32, kind="ExternalInput")
    q_d = nc.dram_tensor("q", [BLOC, K], f32, kind="ExternalOutput")

    with tile.TileContext(nc) as tc:
        with (
            tc.tile_pool(name="xt", bufs=1) as xt_pool,
            tc.tile_pool(name="ct", bufs=NCHUNK) as ct_pool,
            tc.tile_pool(name="psum", bufs=1, space="PSUM") as psum_pool,
            tc.tile_pool(name="dist", bufs=1) as dist_pool,
            tc.tile_pool(name="work", bufs=2) as work_pool,
            tc.tile_pool(name="tail", bufs=1) as tail_pool,
        ):
            xt_t = xt_pool.tile([CP, S * BLOC], fin)
            nc.sync.dma_start(out=xt_t[:], in_=xt_d.ap())
            bias_t = xt_pool.tile([128, 1], f32)
            nc.vector.memset(bias_t[:], 0.0)

            # d2 landing zone: col block t = s//4 (64 wide), partition strip
            # 32*(s%4) + b.  Chunk c covers s in [32c, 32c+32) = bank c.
            psum = psum_pool.tile([128, TBLK * K // 32 * 32], f32)  # [128, 2048]
            # Garbage partitions (16-31 etc.) are never matmul-written; zero
            # them once so the bank-wide sqrt reads defined data.
            nc.vector.memset(psum[:], 0.0)

            # dist, t-major: ACT sqrt writes each psum bank contiguously.
            dist_t = dist_pool.tile([128, TBLK, K], f32)
            # per-chunk partial s-sums, concatenated along free dim
            part4 = tail_pool.tile([128, NCHUNK, K], f32)

            # Spread input DMAs over both HWDGE queues (sync + scalar) so
            # issue latencies overlap; first-needed tensors first.
            ct_tiles = []
            for c in range(NCHUNK):
                ct_t = ct_pool.tile([CP, S_CH * K], fin, tag="ct")
                eng = nc.scalar if c % 2 == 0 else nc.sync
                eng.dma_start(
                    out=ct_t[:], in_=ct_d.ap()[:, c * S_CH * K:(c + 1) * S_CH * K]
                )
                ct_tiles.append(ct_t)
            st_t = xt_pool.tile([128, BLOC], f32)
            nc.scalar.dma_start(out=st_t[:], in_=st_d.ap())

            for c in range(NCHUNK):
                ct_t = ct_tiles[c]
                for u in range(S_CH):
                    s = c * S_CH + u
                    j = s % 4
                    t = s // 4
                    nc.tensor.matmul(
                        psum[32 * j:32 * j + BLOC, t * K:(t + 1) * K],
                        lhsT=xt_t[:, s * BLOC:(s + 1) * BLOC],
                        rhs=ct_t[:, u * K:(u + 1) * K],
                        start=True,
                        stop=True,
                        tile_position=(0, 32 * j),
                    )
                # sqrt of bank c: psum cols [512c, 512c+512) hold t in
                # [8c, 8c+8); contiguous write into dist_t[:, 8c:8c+8, :].
                t0 = 8 * c
                nc.scalar.activation(
                    dist_t[:, t0:t0 + 8, :],
                    psum[:, 512 * c:512 * c + 512],
                    mybir.ActivationFunctionType.Sqrt,
                    bias=bias_t[:],
                )
                nc.vector.tensor_reduce(
                    part4[:, c, :],
                    dist_t[:, t0:t0 + 8, :].rearrange("p t k -> p k t"),
                    axis=mybir.AxisListType.X,
                    op=mybir.AluOpType.add,
                )

            # Strip+partition sum via matmul: out[b, (c,k)] = sum_p strip[p,b]
            # * part4[p, c, k] = sum_j part4[32j+b, c, k].
            dsum_ps = psum_pool.tile([BLOC, NCHUNK * K], f32)
            nc.tensor.matmul(
                dsum_ps[:],
                lhsT=st_t[:],
                rhs=part4[:].rearrange("p c k -> p (c k)"),
                start=True,
                stop=True,
            )
            # dsum[b,k] = sum_c dsum_ps[b, c*K+k]
            dsum = tail_pool.tile([BLOC, K], f32)
            nc.vector.tensor_reduce(
                dsum[:],
                dsum_ps[:].rearrange("p (c k) -> p k c", c=NCHUNK),
                axis=mybir.AxisListType.X,
                op=mybir.AluOpType.add,
            )

            # q tail: w = 1 + dsum^2/2; r = 1/w; r3 = r^3; q = r3/sum_k r3
            w = tail_pool.tile([BLOC, K], f32)
            nc.vector.tensor_tensor(w[:], dsum[:], dsum[:], op=mybir.AluOpType.mult)
            nc.vector.tensor_scalar(
                w[:], w[:], 0.5, 1.0,
                op0=mybir.AluOpType.mult, op1=mybir.AluOpType.add,
            )
            r = tail_pool.tile([BLOC, K], f32)
            nc.vector.reciprocal(r[:], w[:])
            r2 = tail_pool.tile([BLOC, K], f32)
            nc.vector.tensor_tensor(r2[:], r[:], r[:], op=mybir.AluOpType.mult)
            r3 = tail_pool.tile([BLOC, K], f32)
            nc.vector.tensor_tensor(r3[:], r2[:], r[:], op=mybir.AluOpType.mult)
            ssum = tail_pool.tile([BLOC, 1], f32)
            nc.vector.tensor_reduce(
                ssum[:], r3[:], axis=mybir.AxisListType.X, op=mybir.AluOpType.add
            )
            rs = tail_pool.tile([BLOC, 1], f32)
            nc.vector.reciprocal(rs[:], ssum[:])
            qt = tail_pool.tile([BLOC, K], f32)
            nc.vector.tensor_scalar(
                qt[:], r3[:], rs[:], None, op0=mybir.AluOpType.mult
            )
            nc.sync.dma_start(out=q_d.ap(), in_=qt[:])

    nc.compile()
    return nc


def _prep_inputs(x, centroids):
    """Host-side shard + transpose + augmentation. Returns in_maps list."""
    in_np = np.dtype(IN_DT)
    x = np.ascontiguousarray(np.asarray(x, dtype=np.float32)).reshape(B, S, F)
    c = np.ascontiguousarray(np.asarray(centroids, dtype=np.float32)).reshape(K, S, F)

    # CT [66, S*K], cols s*K + k
    ct = np.empty((CP, S * K), dtype=in_np)
    ct[:F] = (-2.0 * c).transpose(2, 1, 0).reshape(F, S * K)
    ct[F] = ((c * c).sum(-1, dtype=np.float32).T).reshape(S * K)
    ct[F + 1] = 1.0

    # strip-sum selector: strip[p, b] = 1 if p % 32 == b else 0
    strip = np.zeros((128, BLOC), dtype=np.float32)
    for p in range(128):
        if p % 32 < BLOC:
            strip[p, p % 32] = 1.0

    in_maps = []
    for i in range(NCORES):
        xs = x[i * BLOC:(i + 1) * BLOC]          # [16, S, F]
        xt = np.empty((CP, S * BLOC), dtype=in_np)
        xt[:F] = xs.transpose(2, 1, 0).reshape(F, S * BLOC)
        xt[F] = 1.0
        xt[F + 1] = ((xs * xs).sum(-1, dtype=np.float32).T).reshape(S * BLOC)
        in_maps.append({"xt": xt, "ct": ct, "strip": strip})
    return in_maps


def kernel(x, centroids):
    from concourse.bass_utils import run_bass_kernel_spmd

    if "nc" not in _CACHE:
        _CACHE["nc"] = _build_nc()
    nc = _CACHE["nc"]

    in_maps = _prep_inputs(x, centroids)
    res = run_bass_kernel_spmd(nc, in_maps, core_ids=list(range(NCORES)))
    out = np.concatenate([res.results[i]["q"] for i in range(NCORES)], axis=0)
    return out.astype(np.float32)


# revision 18
# speedup vs baseline: 1.3406x; 1.0879x over previous
"""Trainium2 Bass kernel for nn_Clustering_80900003987951 (vq_codebook).

Math (reference):
  x: [B=128, S=128, F=64, 1], centroids: [1, K=64, S=128, F=64]
  d2[b,k,s] = sum_f (x[b,s,f] - c[k,s,f])^2
  dist[b,k] = sum_s sqrt(d2[b,k,s])
  q = (1 + dist^2/2)^-3 / 2, normalized over k           -> [B, K]

Strategy: data-parallel over batch across 8 cores (B_loc=16), centroids
replicated. All transposes + augmentation are done host-side:
  XT [66, S*16]: rows 0-63 = x^T (F on partitions), row 64 = 1, row 65 = |x|^2
  CT [66, S*64]: rows 0-63 = -2*c^T,  row 64 = |c|^2,  row 65 = 1
so one fp32 matmul per s gives the complete d2[b,k] tile in PSUM:
  d2 = XT_s^T @ CT_s = -2<x,c> + |c|^2 + |x|^2.
128 matmuls are packed 4-wide with tile_position column tiling; sqrt on
ACT per PSUM bank; s-reduction and the q tail on DVE.
"""

import numpy as np

B, K, S, F = 128, 64, 128, 64
NCORES = 8
BLOC = B // NCORES          # 16
CP = F + 2                  # 66 contraction rows (data + cn + xn aug)
NCHUNK = 4                  # psum banks / s-chunks
S_CH = S // NCHUNK          # 32 s per chunk
TBLK = S // 4               # 32 column blocks of 64 in psum

IN_DT = "float16"   # dtype of the streamed xt/ct operands ("float32"/"float16")

_CACHE = {}


def _build_nc():
    import concourse.bacc as bacc
    import concourse.tile as tile
    from concourse import mybir
    import concourse.bass as bass

    f32 = mybir.dt.float32
    fin = getattr(mybir.dt, IN_DT)
    nc = bacc.Bacc("TRN2", target_bir_lowering=False, debug=False)

    xt_d = nc.dram_tensor("xt", [CP, S * BLOC], fin, kind="ExternalInput")
    ct_d = nc.dram_tensor("ct", [CP, S * K], fin, kind="ExternalInput")
    # strip[p, b] = 1 if p % 32 == b else 0 — matmul-based partition-strip sum
    st_d = nc.dram_tensor("strip", [128, BLOC], f32, kind="ExternalInput")
    q_d = nc.dram_tensor("q", [BLOC, K], f32, kind="ExternalOutput")

    with tile.TileContext(nc) as tc:
        with (
            tc.tile_pool(name="xt", bufs=1) as xt_pool,
            tc.tile_pool(name="ct", bufs=NCHUNK) as ct_pool,
            tc.tile_pool(name="psum", bufs=1, space="PSUM") as psum_pool,
            tc.tile_pool(name="dist", bufs=1) as dist_pool,
            tc.tile_pool(name="work", bufs=2) as work_pool,
            tc.tile_pool(name="tail", bufs=1) as tail_pool,
        ):
            xt_t = xt_pool.tile([CP, S * BLOC], fin)
            nc.sync.dma_start(out=xt_t[:], in_=xt_d.ap())

            # Per-chunk psum tiles (one bank each) and dist tiles so chunk
            # c+1's matmuls don't serialize behind chunk c's sqrt/reduce.
            psums = []
            dists = []
            for c in range(NCHUNK):
                ps = psum_pool.tile([128, 8 * K], f32, name=f"ps{c}", tag=f"ps{c}")
                # Garbage partitions (16-31 etc.) are never matmul-written;
                # zero so the bank-wide sqrt reads defined data.
                nc.vector.memset(ps[:], 0.0)
                psums.append(ps)
                dists.append(
                    dist_pool.tile([128, 8, K], f32, name=f"di{c}", tag=f"di{c}")
                )
            # per-chunk partial s-sums, concatenated along free dim
            part4 = tail_pool.tile([128, NCHUNK, K], f32)

            # Inputs on one queue, in consumption order (engine FIFOs drain
            # in issue order, so ct0 completes first); strip via scalar.
            ct_tiles = []
            for c in range(NCHUNK):
                ct_t = ct_pool.tile([CP, S_CH * K], fin, tag="ct")
                nc.sync.dma_start(
                    out=ct_t[:], in_=ct_d.ap()[:, c * S_CH * K:(c + 1) * S_CH * K]
                )
                ct_tiles.append(ct_t)
            st_t = xt_pool.tile([128, BLOC], f32)
            nc.scalar.dma_start(out=st_t[:], in_=st_d.ap())

            for c in range(NCHUNK):
                ct_t = ct_tiles[c]
                ps = psums[c]
                di = dists[c]
                for u in range(S_CH):
                    s = c * S_CH + u
                    j = s % 4
                    tl = u // 4          # local col block within this bank
                    nc.tensor.matmul(
                        ps[32 * j:32 * j + BLOC, tl * K:(tl + 1) * K],
                        lhsT=xt_t[:, s * BLOC:(s + 1) * BLOC],
                        rhs=ct_t[:, u * K:(u + 1) * K],
                        start=True,
                        stop=True,
                        tile_position=(0, 32 * j),
                    )
                nc.scalar.activation(
                    di[:],
                    ps[:],
                    mybir.ActivationFunctionType.Sqrt,
                )
                nc.vector.tensor_reduce(
                    part4[:, c, :],
                    di[:].rearrange("p t k -> p k t"),
                    axis=mybir.AxisListType.X,
                    op=mybir.AluOpType.add,
                )

            # Strip+partition sum via matmul: out[b, (c,k)] = sum_p strip[p,b]
            # * part4[p, c, k] = sum_j part4[32j+b, c, k].
            dsum_ps = psum_pool.tile([BLOC, NCHUNK * K], f32)
            nc.tensor.matmul(
                dsum_ps[:],
                lhsT=st_t[:],
                rhs=part4[:].rearrange("p c k -> p (c k)"),
                start=True,
                stop=True,
            )
            # dsum[b,k] = sum_c dsum_ps[b, c*K+k]
            dsum = tail_pool.tile([BLOC, K], f32)
            nc.vector.tensor_reduce(
                dsum[:],
                dsum_ps[:].rearrange("p (c k) -> p k c", c=NCHUNK),
                axis=mybir.AxisListType.X,
                op=mybir.AluOpType.add,
            )

            # q tail: w = 1 + dsum^2/2; r = 1/w; r3 = r^3; q = r3/sum_k r3
            w = tail_pool.tile([BLOC, K], f32)
            nc.vector.tensor_tensor(w[:], dsum[:], dsum[:], op=mybir.AluOpType.mult)
            nc.vector.tensor_scalar(
                w[:], w[:], 0.5, 1.0,
                op0=mybir.AluOpType.mult, op1=mybir.AluOpType.add,
            )
            r = tail_pool.tile([BLOC, K], f32)
            nc.vector.reciprocal(r[:], w[:])
            r2 = tail_pool.tile([BLOC, K], f32)
            nc.vector.tensor_tensor(r2[:], r[:], r[:], op=mybir.AluOpType.mult)
            r3 = tail_pool.tile([BLOC, K], f32)
            nc.vector.tensor_tensor(r3[:], r2[:], r[:], op=mybir.AluOpType.mult)
            ssum = tail_pool.tile([BLOC, 1], f32)
            nc.vector.tensor_reduce(
                ssum[:], r3[:], axis=mybir.AxisListType.X, op=mybir.AluOpType.add
            )
            rs = tail_pool.tile([BLOC, 1], f32)
            nc.vector.reciprocal(rs[:], ssum[:])
            qt = tail_pool.tile([BLOC, K], f32)
            nc.vector.tensor_scalar(
                qt[:], r3[:], rs[:], None, op0=mybir.AluOpType.mult
            )
            nc.sync.dma_start(out=q_d.ap(), in_=qt[:])

    nc.compile()
    return nc


def _prep_inputs(x, centroids):
    """Host-side shard + transpose + augmentation. Returns in_maps list."""
    in_np = np.dtype(IN_DT)
    x = np.ascontiguousarray(np.asarray(x, dtype=np.float32)).reshape(B, S, F)
    c = np.ascontiguousarray(np.asarray(centroids, dtype=np.float32)).reshape(K, S, F)

    # CT [66, S*K], cols s*K + k
    ct = np.empty((CP, S * K), dtype=in_np)
    ct[:F] = (-2.0 * c).transpose(2, 1, 0).reshape(F, S * K)
    ct[F] = ((c * c).sum(-1, dtype=np.float32).T).reshape(S * K)
    ct[F + 1] = 1.0

    # strip-sum selector: strip[p, b] = 1 if p % 32 == b else 0
    strip = np.zeros((128, BLOC), dtype=np.float32)
    for p in range(128):
        if p % 32 < BLOC:
            strip[p, p % 32] = 1.0

    in_maps = []
    for i in range(NCORES):
        xs = x[i * BLOC:(i + 1) * BLOC]          # [16, S, F]
        xt = np.empty((CP, S * BLOC), dtype=in_np)
        xt[:F] = xs.transpose(2, 1, 0).reshape(F, S * BLOC)
        xt[F] = 1.0
        xt[F + 1] = ((xs * xs).sum(-1, dtype=np.float32).T).reshape(S * BLOC)
        in_maps.append({"xt": xt, "ct": ct, "strip": strip})
    return in_maps


def kernel(x, centroids):
    from concourse.bass_utils import run_bass_kernel_spmd

    if "nc" not in _CACHE:
        _CACHE["nc"] = _build_nc()
    nc = _CACHE["nc"]

    in_maps = _prep_inputs(x, centroids)
    res = run_bass_kernel_spmd(nc, in_maps, core_ids=list(range(NCORES)))
    out = np.concatenate([res.results[i]["q"] for i in range(NCORES)], axis=0)
    return out.astype(np.float32)


# revision 19
# speedup vs baseline: 1.3978x; 1.0427x over previous
"""Trainium2 Bass kernel for nn_Clustering_80900003987951 (vq_codebook).

Math (reference):
  x: [B=128, S=128, F=64, 1], centroids: [1, K=64, S=128, F=64]
  d2[b,k,s] = sum_f (x[b,s,f] - c[k,s,f])^2
  dist[b,k] = sum_s sqrt(d2[b,k,s])
  q = (1 + dist^2/2)^-3 / 2, normalized over k           -> [B, K]

Strategy: data-parallel over batch across 8 cores (B_loc=16), centroids
replicated. All transposes + augmentation are done host-side:
  XT [66, S*16]: rows 0-63 = x^T (F on partitions), row 64 = 1, row 65 = |x|^2
  CT [66, S*64]: rows 0-63 = -2*c^T,  row 64 = |c|^2,  row 65 = 1
so one fp32 matmul per s gives the complete d2[b,k] tile in PSUM:
  d2 = XT_s^T @ CT_s = -2<x,c> + |c|^2 + |x|^2.
128 matmuls are packed 4-wide with tile_position column tiling; sqrt on
ACT per PSUM bank; s-reduction and the q tail on DVE.
"""

import numpy as np

B, K, S, F = 128, 64, 128, 64
NCORES = 8
BLOC = B // NCORES          # 16
CP = F + 2                  # 66 contraction rows (data + cn + xn aug)
# s-chunks (one psum bank each, <=32 s per bank); smaller final chunks
# shorten the post-DMA serial tail.
CHUNKS = (32, 32, 32, 16, 16)
NCHUNK = len(CHUNKS)

IN_DT = "float16"   # dtype of the streamed xt/ct operands ("float32"/"float16")

_CACHE = {}


def _build_nc():
    import concourse.bacc as bacc
    import concourse.tile as tile
    from concourse import mybir
    import concourse.bass as bass

    f32 = mybir.dt.float32
    fin = getattr(mybir.dt, IN_DT)
    nc = bacc.Bacc("TRN2", target_bir_lowering=False, debug=False)

    xt_d = nc.dram_tensor("xt", [CP, S * BLOC], fin, kind="ExternalInput")
    ct_d = nc.dram_tensor("ct", [CP, S * K], fin, kind="ExternalInput")
    # strip[p, b] = 1 if p % 32 == b else 0 — matmul-based partition-strip sum
    st_d = nc.dram_tensor("strip", [128, BLOC], f32, kind="ExternalInput")
    q_d = nc.dram_tensor("q", [BLOC, K], f32, kind="ExternalOutput")

    with tile.TileContext(nc) as tc:
        with (
            tc.tile_pool(name="xt", bufs=1) as xt_pool,
            tc.tile_pool(name="ct", bufs=NCHUNK) as ct_pool,
            tc.tile_pool(name="psum", bufs=1, space="PSUM") as psum_pool,
            tc.tile_pool(name="dist", bufs=1) as dist_pool,
            tc.tile_pool(name="work", bufs=2) as work_pool,
            tc.tile_pool(name="tail", bufs=1) as tail_pool,
        ):
            xt_t = xt_pool.tile([CP, S * BLOC], fin)
            nc.sync.dma_start(out=xt_t[:], in_=xt_d.ap())

            # Per-chunk psum tiles (one bank each) and dist tiles so chunk
            # c+1's matmuls don't serialize behind chunk c's sqrt/reduce.
            psums = []
            dists = []
            for c, csz in enumerate(CHUNKS):
                nt = csz // 4
                ps = psum_pool.tile([128, nt * K], f32, name=f"ps{c}", tag=f"ps{c}")
                # Garbage partitions (16-31 etc.) are never matmul-written;
                # zero so the bank-wide sqrt reads defined data.
                nc.vector.memset(ps[:], 0.0)
                psums.append(ps)
                dists.append(
                    dist_pool.tile([128, nt, K], f32, name=f"di{c}", tag=f"di{c}")
                )
            # cross-partition strip sums accumulate here via matmul
            dsum_ps = psum_pool.tile([BLOC, K], f32)

            # Inputs on one queue, in consumption order (engine FIFOs drain
            # in issue order, so ct0 completes first); strip via scalar.
            ct_tiles = []
            off = 0
            for c, csz in enumerate(CHUNKS):
                ct_t = ct_pool.tile([CP, csz * K], fin, tag=f"ct{c}")
                nc.sync.dma_start(
                    out=ct_t[:], in_=ct_d.ap()[:, off * K:(off + csz) * K]
                )
                ct_tiles.append(ct_t)
                off += csz
            st_t = xt_pool.tile([128, BLOC], f32)
            nc.scalar.dma_start(out=st_t[:], in_=st_d.ap())

            soff = 0
            for c, csz in enumerate(CHUNKS):
                ct_t = ct_tiles[c]
                ps = psums[c]
                di = dists[c]
                for u in range(csz):
                    s = soff + u
                    j = s % 4
                    tl = u // 4          # local col block within this bank
                    nc.tensor.matmul(
                        ps[32 * j:32 * j + BLOC, tl * K:(tl + 1) * K],
                        lhsT=xt_t[:, s * BLOC:(s + 1) * BLOC],
                        rhs=ct_t[:, u * K:(u + 1) * K],
                        start=True,
                        stop=True,
                        tile_position=(0, 32 * j),
                    )
                soff += csz
                nc.scalar.activation(
                    di[:],
                    ps[:],
                    mybir.ActivationFunctionType.Sqrt,
                )
                part = work_pool.tile([128, K], f32, tag="part")
                nc.vector.tensor_reduce(
                    part[:],
                    di[:].rearrange("p t k -> p k t"),
                    axis=mybir.AxisListType.X,
                    op=mybir.AluOpType.add,
                )
                # Accumulate the strip/partition sum into psum: after the
                # last chunk dsum_ps[b,k] = sum_s dist[b,k,s].
                nc.tensor.matmul(
                    dsum_ps[:],
                    lhsT=st_t[:],
                    rhs=part[:],
                    start=(c == 0),
                    stop=(c == NCHUNK - 1),
                    skip_group_check=True,
                )

            # q tail: w = 1 + dsum^2/2; r = 1/w; r3 = r^3; q = r3/sum_k r3
            dsum = tail_pool.tile([BLOC, K], f32)
            nc.scalar.activation(
                dsum[:], dsum_ps[:], mybir.ActivationFunctionType.Copy
            )
            w = tail_pool.tile([BLOC, K], f32)
            nc.vector.tensor_tensor(w[:], dsum[:], dsum[:], op=mybir.AluOpType.mult)
            nc.vector.tensor_scalar(
                w[:], w[:], 0.5, 1.0,
                op0=mybir.AluOpType.mult, op1=mybir.AluOpType.add,
            )
            r = tail_pool.tile([BLOC, K], f32)
            nc.vector.reciprocal(r[:], w[:])
            r2 = tail_pool.tile([BLOC, K], f32)
            nc.vector.tensor_tensor(r2[:], r[:], r[:], op=mybir.AluOpType.mult)
            r3 = tail_pool.tile([BLOC, K], f32)
            nc.vector.tensor_tensor(r3[:], r2[:], r[:], op=mybir.AluOpType.mult)
            ssum = tail_pool.tile([BLOC, 1], f32)
            nc.vector.tensor_reduce(
                ssum[:], r3[:], axis=mybir.AxisListType.X, op=mybir.AluOpType.add
            )
            rs = tail_pool.tile([BLOC, 1], f32)
            nc.vector.reciprocal(rs[:], ssum[:])
            qt = tail_pool.tile([BLOC, K], f32)
            nc.vector.tensor_scalar(
                qt[:], r3[:], rs[:], None, op0=mybir.AluOpType.mult
            )
            nc.sync.dma_start(out=q_d.ap(), in_=qt[:])

    nc.compile()
    return nc


def _prep_inputs(x, centroids):
    """Host-side shard + transpose + augmentation. Returns in_maps list."""
    in_np = np.dtype(IN_DT)
    x = np.ascontiguousarray(np.asarray(x, dtype=np.float32)).reshape(B, S, F)
    c = np.ascontiguousarray(np.asarray(centroids, dtype=np.float32)).reshape(K, S, F)

    # CT [66, S*K], cols s*K + k
    ct = np.empty((CP, S * K), dtype=in_np)
    ct[:F] = (-2.0 * c).transpose(2, 1, 0).reshape(F, S * K)
    ct[F] = ((c * c).sum(-1, dtype=np.float32).T).reshape(S * K)
    ct[F + 1] = 1.0

    # strip-sum selector: strip[p, b] = 1 if p % 32 == b else 0
    strip = np.zeros((128, BLOC), dtype=np.float32)
    for p in range(128):
        if p % 32 < BLOC:
            strip[p, p % 32] = 1.0

    in_maps = []
    for i in range(NCORES):
        xs = x[i * BLOC:(i + 1) * BLOC]          # [16, S, F]
        xt = np.empty((CP, S * BLOC), dtype=in_np)
        xt[:F] = xs.transpose(2, 1, 0).reshape(F, S * BLOC)
        xt[F] = 1.0
        xt[F + 1] = ((xs * xs).sum(-1, dtype=np.float32).T).reshape(S * BLOC)
        in_maps.append({"xt": xt, "ct": ct, "strip": strip})
    return in_maps


def kernel(x, centroids):
    from concourse.bass_utils import run_bass_kernel_spmd

    if "nc" not in _CACHE:
        _CACHE["nc"] = _build_nc()
    nc = _CACHE["nc"]

    in_maps = _prep_inputs(x, centroids)
    res = run_bass_kernel_spmd(nc, in_maps, core_ids=list(range(NCORES)))
    out = np.concatenate([res.results[i]["q"] for i in range(NCORES)], axis=0)
    return out.astype(np.float32)


# revision 20
# speedup vs baseline: 1.4710x; 1.0523x over previous
"""Trainium2 Bass kernel for nn_Clustering_80900003987951 (vq_codebook).

Math (reference):
  x: [B=128, S=128, F=64, 1], centroids: [1, K=64, S=128, F=64]
  d2[b,k,s] = sum_f (x[b,s,f] - c[k,s,f])^2
  dist[b,k] = sum_s sqrt(d2[b,k,s])
  q = (1 + dist^2/2)^-3 / 2, normalized over k           -> [B, K]

Strategy: data-parallel over batch across 8 cores (B_loc=16), centroids
replicated. All transposes + augmentation are done host-side:
  XT [66, S*16]: rows 0-63 = x^T (F on partitions), row 64 = 1, row 65 = |x|^2
  CT [66, S*64]: rows 0-63 = -2*c^T,  row 64 = |c|^2,  row 65 = 1
so one fp32 matmul per s gives the complete d2[b,k] tile in PSUM:
  d2 = XT_s^T @ CT_s = -2<x,c> + |c|^2 + |x|^2.
128 matmuls are packed 4-wide with tile_position column tiling; sqrt on
ACT per PSUM bank; s-reduction and the q tail on DVE.
"""

import numpy as np

B, K, S, F = 128, 64, 128, 64
NCORES = 8
BLOC = B // NCORES          # 16
CP = F + 2                  # 66 contraction rows (data + cn + xn aug)
# s-chunks (one psum bank each, <=32 s per bank); smaller final chunks
# shorten the post-DMA serial tail.
CHUNKS = (32, 32, 32, 16, 16)
NCHUNK = len(CHUNKS)

XT_DT = "float16"   # dtype of the streamed xt operand
CT_DT = "float8e4"  # dtype of the big replicated ct operand (fp8 e4m3)

_CACHE = {}


def _build_nc():
    import concourse.bacc as bacc
    import concourse.tile as tile
    from concourse import mybir
    import concourse.bass as bass

    f32 = mybir.dt.float32
    fxt = getattr(mybir.dt, XT_DT)
    fct = getattr(mybir.dt, CT_DT)
    nc = bacc.Bacc("TRN2", target_bir_lowering=False, debug=False)

    xt_d = nc.dram_tensor("xt", [CP, S * BLOC], fxt, kind="ExternalInput")
    ct_d = nc.dram_tensor("ct", [CP, S * K], fct, kind="ExternalInput")
    # strip[p, b] = 1 if p % 32 == b else 0 — matmul-based partition-strip sum
    st_d = nc.dram_tensor("strip", [128, BLOC], f32, kind="ExternalInput")
    q_d = nc.dram_tensor("q", [BLOC, K], f32, kind="ExternalOutput")

    with tile.TileContext(nc) as tc:
        with (
            tc.tile_pool(name="xt", bufs=1) as xt_pool,
            tc.tile_pool(name="ct", bufs=NCHUNK) as ct_pool,
            tc.tile_pool(name="psum", bufs=1, space="PSUM") as psum_pool,
            tc.tile_pool(name="dist", bufs=1) as dist_pool,
            tc.tile_pool(name="work", bufs=2) as work_pool,
            tc.tile_pool(name="tail", bufs=1) as tail_pool,
        ):
            xt_t = xt_pool.tile([CP, S * BLOC], fxt)
            nc.sync.dma_start(out=xt_t[:], in_=xt_d.ap())

            # Per-chunk psum tiles (one bank each) and dist tiles so chunk
            # c+1's matmuls don't serialize behind chunk c's sqrt/reduce.
            psums = []
            dists = []
            for c, csz in enumerate(CHUNKS):
                nt = csz // 4
                ps = psum_pool.tile([128, nt * K], f32, name=f"ps{c}", tag=f"ps{c}")
                # Garbage partitions (16-31 etc.) are never matmul-written;
                # zero so the bank-wide sqrt reads defined data.
                nc.vector.memset(ps[:], 0.0)
                psums.append(ps)
                dists.append(
                    dist_pool.tile([128, nt, K], f32, name=f"di{c}", tag=f"di{c}")
                )
            # cross-partition strip sums accumulate here via matmul
            dsum_ps = psum_pool.tile([BLOC, K], f32)

            # Inputs on one queue, in consumption order (engine FIFOs drain
            # in issue order, so ct0 completes first); strip via scalar.
            ct_tiles = []
            off = 0
            for c, csz in enumerate(CHUNKS):
                ct_t = ct_pool.tile([CP, csz * K], fct, tag=f"ct{c}")
                nc.sync.dma_start(
                    out=ct_t[:], in_=ct_d.ap()[:, off * K:(off + csz) * K]
                )
                ct_tiles.append(ct_t)
                off += csz
            st_t = xt_pool.tile([128, BLOC], f32)
            nc.scalar.dma_start(out=st_t[:], in_=st_d.ap())

            soff = 0
            for c, csz in enumerate(CHUNKS):
                ct_t = ct_tiles[c]
                ps = psums[c]
                di = dists[c]
                for u in range(csz):
                    s = soff + u
                    j = s % 4
                    tl = u // 4          # local col block within this bank
                    nc.tensor.matmul(
                        ps[32 * j:32 * j + BLOC, tl * K:(tl + 1) * K],
                        lhsT=xt_t[:, s * BLOC:(s + 1) * BLOC],
                        rhs=ct_t[:, u * K:(u + 1) * K],
                        start=True,
                        stop=True,
                        tile_position=(0, 32 * j),
                    )
                soff += csz
                nc.scalar.activation(
                    di[:],
                    ps[:],
                    mybir.ActivationFunctionType.Sqrt,
                )
                part = work_pool.tile([128, K], f32, tag="part")
                nc.vector.tensor_reduce(
                    part[:],
                    di[:].rearrange("p t k -> p k t"),
                    axis=mybir.AxisListType.X,
                    op=mybir.AluOpType.add,
                )
                # Accumulate the strip/partition sum into psum: after the
                # last chunk dsum_ps[b,k] = sum_s dist[b,k,s].
                nc.tensor.matmul(
                    dsum_ps[:],
                    lhsT=st_t[:],
                    rhs=part[:],
                    start=(c == 0),
                    stop=(c == NCHUNK - 1),
                    skip_group_check=True,
                )

            # q tail: w = 1 + dsum^2/2; r = 1/w; r3 = r^3; q = r3/sum_k r3
            dsum = tail_pool.tile([BLOC, K], f32)
            nc.scalar.activation(
                dsum[:], dsum_ps[:], mybir.ActivationFunctionType.Copy
            )
            w = tail_pool.tile([BLOC, K], f32)
            nc.vector.tensor_tensor(w[:], dsum[:], dsum[:], op=mybir.AluOpType.mult)
            nc.vector.tensor_scalar(
                w[:], w[:], 0.5, 1.0,
                op0=mybir.AluOpType.mult, op1=mybir.AluOpType.add,
            )
            r = tail_pool.tile([BLOC, K], f32)
            nc.vector.reciprocal(r[:], w[:])
            r2 = tail_pool.tile([BLOC, K], f32)
            nc.vector.tensor_tensor(r2[:], r[:], r[:], op=mybir.AluOpType.mult)
            r3 = tail_pool.tile([BLOC, K], f32)
            nc.vector.tensor_tensor(r3[:], r2[:], r[:], op=mybir.AluOpType.mult)
            ssum = tail_pool.tile([BLOC, 1], f32)
            nc.vector.tensor_reduce(
                ssum[:], r3[:], axis=mybir.AxisListType.X, op=mybir.AluOpType.add
            )
            rs = tail_pool.tile([BLOC, 1], f32)
            nc.vector.reciprocal(rs[:], ssum[:])
            qt = tail_pool.tile([BLOC, K], f32)
            nc.vector.tensor_scalar(
                qt[:], r3[:], rs[:], None, op0=mybir.AluOpType.mult
            )
            nc.sync.dma_start(out=q_d.ap(), in_=qt[:])

    nc.compile()
    return nc


def _prep_inputs(x, centroids):
    """Host-side shard + transpose + augmentation. Returns in_maps list."""
    from concourse import mybir

    xt_np = mybir.dt.np(getattr(mybir.dt, XT_DT))
    ct_np = mybir.dt.np(getattr(mybir.dt, CT_DT))
    x = np.ascontiguousarray(np.asarray(x, dtype=np.float32)).reshape(B, S, F)
    c = np.ascontiguousarray(np.asarray(centroids, dtype=np.float32)).reshape(K, S, F)

    # CT [66, S*K], cols s*K + k
    ct = np.empty((CP, S * K), dtype=ct_np)
    ct[:F] = (-2.0 * c).transpose(2, 1, 0).reshape(F, S * K)
    ct[F] = ((c * c).sum(-1, dtype=np.float32).T).reshape(S * K)
    ct[F + 1] = 1.0

    # strip-sum selector: strip[p, b] = 1 if p % 32 == b else 0
    strip = np.zeros((128, BLOC), dtype=np.float32)
    for p in range(128):
        if p % 32 < BLOC:
            strip[p, p % 32] = 1.0

    in_maps = []
    for i in range(NCORES):
        xs = x[i * BLOC:(i + 1) * BLOC]          # [16, S, F]
        xt = np.empty((CP, S * BLOC), dtype=xt_np)
        xt[:F] = xs.transpose(2, 1, 0).reshape(F, S * BLOC)
        xt[F] = 1.0
        xt[F + 1] = ((xs * xs).sum(-1, dtype=np.float32).T).reshape(S * BLOC)
        in_maps.append({"xt": xt, "ct": ct, "strip": strip})
    return in_maps


def kernel(x, centroids):
    from concourse.bass_utils import run_bass_kernel_spmd

    if "nc" not in _CACHE:
        _CACHE["nc"] = _build_nc()
    nc = _CACHE["nc"]

    in_maps = _prep_inputs(x, centroids)
    res = run_bass_kernel_spmd(nc, in_maps, core_ids=list(range(NCORES)))
    out = np.concatenate([res.results[i]["q"] for i in range(NCORES)], axis=0)
    return out.astype(np.float32)
